# revision 16
# baseline (speedup 1.0000x reference)
"""Trainium2 Bass kernel: contrastive (NT-Xent style) loss over cosine
similarities.

loss = -mean_i log( sum_j(exp(cos_ij/tau) * pos_ij) / (sum_j exp(cos_ij/tau) + 1e-8) )

Sharding: rows of z are split across 8 NeuronCores (data parallel over N).
Each core computes its [N/8, N] block of the similarity matrix against the
full (all-rows) z, flash-style in [128, 512] tiles, reducing to per-row
S_i = sum_j exp(c_ij) and P_i = sum_j exp(c_ij) * pos_ij, then
sum_i (ln(S_i + eps) - ln(P_i)).  The host sums the 8 per-core partials.

Device pipeline per core:
  - normalize z rows: ssq via fused square+row-sum, 1/sqrt, then the
    normalization is folded into the PE transpose as a diag(rn) stationary
    operand (out = z_chunk^T @ diag(rn)) -> normalized z^T in SBUF.
  - main loop over (j_tile, m_block): 4 accumulating float32r matmuls
    (K=128 d-chunks) -> PSUM;  ScalarE Exp(scale=1/tau) with fused
    per-partition row-sum accumulation (S);  DVE tensor_tensor_reduce
    (E * pos, fused row-sum) for P, partially offloaded to GPSIMD.
  - epilogue: ln(S+eps) - ln(P), reduce over rows, partition-reduce on
    GPSIMD, DMA one fp32 scalar out.
"""

import numpy as np
from contextlib import ExitStack

N = 8192
D = 512
NCORES = 8
RPC = N // NCORES  # rows per core
TAU = 0.8
INV_TAU = 1.0 / TAU
EPS = 1e-8

PART = 128       # SBUF partitions
JT = 512         # j-tile width (moving dim of matmul)
GRP = 8          # n-chunks per PSUM->SBUF copy group in transpose setup


def _emit(nc, tc, ctx, z_ap, zm_ap, pos_ap, out_ap, n, d, rpc):
    import concourse.mybir as mybir

    f32 = mybir.dt.float32
    bf16 = mybir.dt.bfloat16
    i32 = mybir.dt.int32
    ALU = mybir.AluOpType
    ACT = mybir.ActivationFunctionType
    AX = mybir.AxisListType

    nch = n // PART        # chunks on the all-rows side (64)
    mch = rpc // PART      # chunks on this core's row-block side (8)
    dq = d // PART         # contraction sub-tiles, K=128 each (4)
    JT2 = 2 * JT           # 1024-wide elementwise tiles
    njt2 = n // JT2        # 8
    nsc = 2 * njt2         # 16 scol columns
    assert GRP == 8 and nch % GRP == 0 and mch == GRP

    const_pool = ctx.enter_context(tc.tile_pool(name="const", bufs=1))
    big_pool = ctx.enter_context(tc.tile_pool(name="big", bufs=1))
    zin_pool = ctx.enter_context(tc.tile_pool(name="zin", bufs=4))
    sq_pool = ctx.enter_context(tc.tile_pool(name="sq", bufs=2))
    small_pool = ctx.enter_context(tc.tile_pool(name="small", bufs=2))
    zcn_pool = ctx.enter_context(tc.tile_pool(name="zcn", bufs=3))
    e_pool = ctx.enter_context(tc.tile_pool(name="epool", bufs=4))
    pos_pool = ctx.enter_context(tc.tile_pool(name="pospool", bufs=4))
    ttr_pool = ctx.enter_context(tc.tile_pool(name="ttro", bufs=2))
    acc_pool = ctx.enter_context(tc.tile_pool(name="accp", bufs=1))
    tp_psum = ctx.enter_context(tc.tile_pool(name="tpp", bufs=1, space="PSUM"))
    mm_psum = ctx.enter_context(tc.tile_pool(name="mmp", bufs=4, space="PSUM"))

    # --- constants ---
    idx = const_pool.tile([PART, PART], i32, name="idx", tag="idx")
    nc.gpsimd.iota(idx[:], pattern=[[1, PART]], base=0, channel_multiplier=-1)
    ident = const_pool.tile([PART, PART], bf16, name="ident", tag="ident")
    nc.vector.tensor_scalar(ident[:], idx[:], 0, None, ALU.is_equal)
    epst = const_pool.tile([PART, 1], f32, name="epst", tag="epst")
    nc.vector.memset(epst[:], EPS)

    # --- persistent transposed-normalized operands (bf16) ---
    zhT = [
        big_pool.tile([PART, n], bf16, name=f"zhT{q}", tag=f"zhT{q}")
        for q in range(dq)
    ]
    zmT = [
        big_pool.tile([PART, rpc], bf16, name=f"zmT{q}", tag=f"zmT{q}")
        for q in range(dq)
    ]
    # row-sum collectors
    scol = [
        acc_pool.tile([PART, nsc], f32, name=f"scol{mb}", tag=f"scol{mb}")
        for mb in range(mch)
    ]
    pcol = [
        acc_pool.tile([PART, njt2], f32, name=f"pcol{mb}", tag=f"pcol{mb}")
        for mb in range(mch)
    ]
    lcol = acc_pool.tile([PART, mch], f32, name="lcol", tag="lcol")
    # 1/||z_row|| per chunk, [128, n_chunks]
    rn_m = acc_pool.tile([PART, mch], f32, name="rn_m", tag="rn_m")
    rn_z = acc_pool.tile([PART, nch], f32, name="rn_z", tag="rn_z")
    ssq_m = acc_pool.tile([PART, mch], f32, name="ssq_m", tag="ssq_m")
    ssq_z = acc_pool.tile([PART, nch], f32, name="ssq_z", tag="ssq_z")
    rs_m = acc_pool.tile([PART, mch], f32, name="rs_m", tag="rs_m")
    rs_z = acc_pool.tile([PART, nch], f32, name="rs_z", tag="rs_z")

    def norm_batch(src_ap, chunks, ssqt, rst, rnt, who):
        """Row sum-of-squares for a batch of 128-row chunks, then
        rsqrt via one batched Sqrt + reciprocal."""
        for k, c in enumerate(chunks):
            zc = zin_pool.tile([PART, d], f32, name=f"za{who}{c}", tag="zc")
            nc.sync.dma_start(out=zc[:], in_=src_ap[PART * c:PART * (c + 1), :])
            sqt = sq_pool.tile([PART, d], bf16, name=f"sq{who}{c}", tag="sqt")
            if k % 2 == 0:
                nc.scalar.activation(
                    sqt[:], zc[:], ACT.Square, accum_out=ssqt[:, c:c + 1]
                )
            else:
                nc.vector.scalar_tensor_tensor(
                    out=sqt[:], in0=zc[:], scalar=0.0, in1=zc[:],
                    op0=ALU.bypass, op1=ALU.mult, accum_out=ssqt[:, c:c + 1],
                )
        lo, hi = chunks[0], chunks[-1] + 1
        nc.scalar.activation(rst[:, lo:hi], ssqt[:, lo:hi], ACT.Sqrt)
        nc.vector.reciprocal(rnt[:, lo:hi], rst[:, lo:hi])

    def setup_group(src_ap, g, dstT, rnt, who):
        """Normalize + transpose GRP chunks of 128 rows into dstT (bf16)."""
        ptiles = [
            tp_psum.tile(
                [PART, PART * GRP], bf16, name=f"tp{who}{g}q{q}", tag=f"tp{q}"
            )
            for q in range(dq)
        ]
        for cc in range(GRP):
            c = g * GRP + cc
            zc = zin_pool.tile([PART, d], f32, name=f"zb{who}{c}", tag="zc")
            nc.sync.dma_start(out=zc[:], in_=src_ap[PART * c:PART * (c + 1), :])
            zcn = zcn_pool.tile([PART, d], bf16, name=f"zcn{who}{c}", tag="zcn")
            nc.vector.tensor_scalar(zcn[:], zc[:], rnt[:, c:c + 1], None, ALU.mult)
            for q in range(dq):
                nc.tensor.transpose(
                    out=ptiles[q][:, PART * cc:PART * (cc + 1)],
                    in_=zcn[:, PART * q:PART * (q + 1)],
                    identity=ident[:],
                )
        for q in range(dq):
            dst = dstT[q][:, PART * GRP * g:PART * GRP * (g + 1)]
            if q % 2 == 0:
                nc.scalar.copy(dst, ptiles[q][:])
            else:
                nc.vector.tensor_copy(dst, ptiles[q][:])

    def main_tile(jt2, mb):
        pss = []
        for h in range(2):
            ps = mm_psum.tile([PART, JT], f32, name=f"ps{jt2}_{mb}_{h}", tag="ps")
            for q in range(dq):
                nc.tensor.matmul(
                    out=ps[:],
                    lhsT=zmT[q][:, PART * mb:PART * (mb + 1)],
                    rhs=zhT[q][:, JT2 * jt2 + JT * h:JT2 * jt2 + JT * (h + 1)],
                    start=(q == 0),
                    stop=(q == dq - 1),
                )
            pss.append(ps)
        et = e_pool.tile([PART, JT2], bf16, name=f"e{jt2}_{mb}", tag="et")
        for h in range(2):
            nc.scalar.activation(
                et[:, JT * h:JT * (h + 1)], pss[h][:], ACT.Exp, scale=INV_TAU,
                accum_out=scol[mb][:, 2 * jt2 + h:2 * jt2 + h + 1],
            )
        pt = pos_pool.tile([PART, JT2], bf16, name=f"p{jt2}_{mb}", tag="pt")
        nc.sync.dma_start(
            out=pt[:],
            in_=pos_ap[PART * mb:PART * (mb + 1), JT2 * jt2:JT2 * (jt2 + 1)],
        )
        to = ttr_pool.tile([PART, JT2], bf16, name=f"t{jt2}_{mb}", tag="to")
        nc.vector.scalar_tensor_tensor(
            out=to[:], in0=et[:], scalar=0.0, in1=pt[:],
            op0=ALU.bypass, op1=ALU.mult,
            accum_out=pcol[mb][:, jt2:jt2 + 1],
        )

    # --- phase A: norms (this core's block, then first z groups) ---
    norm_batch(zm_ap, list(range(mch)), ssq_m, rs_m, rn_m, "m")
    norm_batch(z_ap, list(range(0, GRP * 2)), ssq_z, rs_z, rn_z, "z")

    # --- stationary operands ---
    setup_group(zm_ap, 0, zmT, rn_m, "m")

    # --- interleaved: z-transpose group jt2 feeds main tiles of jt2 ---
    for jt2 in range(njt2):
        if jt2 + 2 < njt2:
            norm_batch(
                z_ap, list(range(GRP * (jt2 + 2), GRP * (jt2 + 3))),
                ssq_z, rs_z, rn_z, "z",
            )
        setup_group(z_ap, jt2, zhT, rn_z, "z")
        for mb in range(mch):
            main_tile(jt2, mb)

    # --- epilogue ---
    for mb in range(mch):
        sm = small_pool.tile([PART, 1], f32, name=f"sm{mb}", tag="sm")
        nc.vector.tensor_reduce(sm[:], scol[mb][:], AX.X, ALU.add)
        pm = small_pool.tile([PART, 1], f32, name=f"pm{mb}", tag="pm")
        nc.vector.tensor_reduce(pm[:], pcol[mb][:], AX.X, ALU.add)
        ls = small_pool.tile([PART, 1], f32, name=f"ls{mb}", tag="ls")
        nc.scalar.activation(ls[:], sm[:], ACT.Ln, bias=epst[:])
        lp = small_pool.tile([PART, 1], f32, name=f"lp{mb}", tag="lp")
        nc.scalar.activation(lp[:], pm[:], ACT.Ln)
        nc.vector.tensor_sub(lcol[:, mb:mb + 1], ls[:], lp[:])

    lsum = small_pool.tile([PART, 1], f32, name="lsum", tag="lsum")
    nc.vector.tensor_reduce(lsum[:], lcol[:], AX.X, ALU.add)
    nc.sync.dma_start(out=out_ap[:, :], in_=lsum[:])


def _build(n=N, d=D, rpc=RPC):
    import concourse.bacc as bacc
    import concourse.tile as tile
    import concourse.mybir as mybir

    f32 = mybir.dt.float32
    bf16 = mybir.dt.bfloat16

    nc = bacc.Bacc(trn_type="TRN2", target_bir_lowering=False, debug=False)
    z_ap = nc.dram_tensor("z", [n, d], f32, kind="ExternalInput").ap()
    zm_ap = nc.dram_tensor("zm", [rpc, d], f32, kind="ExternalInput").ap()
    pos_ap = nc.dram_tensor("posb", [rpc, n], bf16, kind="ExternalInput").ap()
    out_ap = nc.dram_tensor("out", [PART, 1], f32, kind="ExternalOutput").ap()

    with tile.TileContext(nc) as tc:
        with ExitStack() as ctx:
            _emit(nc, tc, ctx, z_ap, zm_ap, pos_ap, out_ap, n, d, rpc)
    nc.compile()
    return nc


_NC_CACHE = {}


def _get_nc():
    if "nc" not in _NC_CACHE:
        _NC_CACHE["nc"] = _build()
    return _NC_CACHE["nc"]


def _make_in_maps(z, pos):
    import ml_dtypes

    z = np.ascontiguousarray(np.asarray(z, dtype=np.float32))
    pos = np.asarray(pos)
    posb = pos.astype(ml_dtypes.bfloat16)
    in_maps = []
    for r in range(NCORES):
        lo, hi = r * RPC, (r + 1) * RPC
        in_maps.append(
            {
                "z": z,
                "zm": np.ascontiguousarray(z[lo:hi]),
                "posb": np.ascontiguousarray(posb[lo:hi]),
            }
        )
    return in_maps


def _run(z, pos, trace=False):
    from concourse.bass_utils import run_bass_kernel_spmd

    nc = _get_nc()
    in_maps = _make_in_maps(z, pos)
    res = run_bass_kernel_spmd(
        nc, in_maps, core_ids=list(range(NCORES)), trace=trace
    )
    partials = np.array(
        [res.results[r]["out"].astype(np.float64).sum() for r in range(NCORES)]
    )
    loss = partials.sum() / N
    return np.asarray(loss, dtype=np.float32), res


def kernel(z, pos):
    out, _ = _run(z, pos, trace=False)
    return out
